# revision 1
# baseline (speedup 1.0000x reference)
"""IoU metric kernel for Trainium2 (Bass/Tile), 8-core data-parallel over batch.

Problem: input [16,21,512,512] f32 logits, target [16,21,512,512] f32 0/1 masks.
  pred = argmax_C(input); per-(b,c): inter = sum(target * onehot(pred)),
  gt = sum(target), pr = sum(onehot(pred)); present = any(target) = (gt > 0).
  scores[c] = (sum_b present*inter) / (sum_b present*(gt+pr) - inter_s + eps) * counts
Returns (scores[1:], counts[1:]).

Sharding: batch 16 -> 8 cores x 2 images. Host combines per-image partials.

v6 design (fused one-pass PE reduction; DVE critical path balanced with Pool):
  - Host casts input f32 -> fp16 (argmax ties from quantization cost ~1.1e-3
    rel err, gate is 2e-2; HW-verified) and target -> fp8e4m3 (exact for 0/1).
    DMA per core: 22 MB + 11 MB = 33 MB -> ~95 us at the model's 360 GB/s.
  - Per 128-row chunk: a 7-instruction pairwise-max tree (fp16 tensor_tensor,
    2x_1p DVE mode) then one-hot planes via is_equal against a stride-0
    broadcast of the max. Pool cannot run max/is_equal TensorTensor on TRN2
    (codegen engine-check), but it CAN run subtract + tensor_scalar, so 5 of
    the 21 one-hot planes compute on Pool as (x - m) then ==0.
  - Both the target tile and the one-hot tile carry a 22nd all-ones plane
    (written once per buffer at startup). One matmul per w column,
    psum[22,22] += t_aug[:,:,w].T @ oh_aug[:,:,w], accumulates ALL THREE
    quantities at once: diag = inter, row 21 = pr (ones.T @ oh), col 21 = gt
    (t.T @ ones). No selector matmuls, no separate multiply, no reduces.
  - Schedule: target DMAs are deferred one chunk so the last input chunk
    lands ~4 us earlier; the last chunk skips the Pool offload (cross-engine
    tail) and splits its one-hot + matmuls by w-halves so PE overlaps the
    final DVE work.
  - PSUM [22,22] f32 is copied to SBUF and DMA'd out per image; host applies
    the reference formula in f64.
"""

import threading

import numpy as np

import concourse.bacc as bacc
import concourse.mybir as mybir
import concourse.tile as tile
from concourse.alu_op_type import AluOpType
from concourse.bass_utils import run_bass_kernel_spmd

F32 = mybir.dt.float32
F16 = mybir.dt.float16
F8 = mybir.dt.float8e4

B, C, H, W = 16, 21, 512, 512
NCORES = 8
BPC = B // NCORES  # images per core
P = 128
CA = C + 1  # classes + ones plane
SD = CA  # stats dim = 22
POOL_EQ = 5  # one-hot planes computed on Pool (as sub + ==0)
NXB = 3  # input-tile buffers


def build_kernel_ir(nc, bpc=BPC):
    chunks = H // P  # h-row chunks per image
    tt, eq, sub = AluOpType.max, AluOpType.is_equal, AluOpType.subtract

    inp = nc.dram_tensor("input", [bpc, C, H, W], F16, kind="ExternalInput")
    tgt = nc.dram_tensor("target", [bpc, C, H, W], F8, kind="ExternalInput")
    stats = nc.dram_tensor("stats", [bpc, SD, SD], F32, kind="ExternalOutput")

    inp_r = inp.ap().rearrange("b c (j p) w -> b j p c w", p=P)
    tgt_r = tgt.ap().rearrange("b c (j p) w -> b j p c w", p=P)
    stats_ap = stats.ap()
    nflat = bpc * chunks

    with tile.TileContext(nc) as tc:
        with tc.tile_pool(name="data", bufs=1) as dp, \
             tc.tile_pool(name="psum", bufs=1, space="PSUM") as pp:
            xb = [dp.tile([P, C, W], F16, tag=f"xb{i}", name=f"xb{i}") for i in range(NXB)]
            tb = [dp.tile([P, CA, W], F8, tag=f"tb{i}", name=f"tb{i}") for i in range(2)]
            oh = [dp.tile([P, CA, W], F16, tag=f"oh{i}", name=f"oh{i}") for i in range(2)]
            scr = [dp.tile([P, 10, W], F16, tag=f"scr{i}", name=f"scr{i}") for i in range(2)]
            m = [dp.tile([P, W], F16, tag=f"m{i}", name=f"m{i}") for i in range(2)]
            pscr = [dp.tile([P, POOL_EQ, W], F16, tag=f"pscr{i}", name=f"pscr{i}") for i in range(1)]
            res = [dp.tile([SD, SD], F32, tag=f"res{i}", name=f"res{i}") for i in range(bpc)]

            # ones planes: written once; DMA / one-hot writes never touch them
            for i in range(2):
                nc.vector.memset(tb[i][:, C, :], 1.0)
                nc.vector.memset(oh[i][:, C, :], 1.0)

            psums = [
                pp.tile([SD, SD], F32, tag=f"ps{i}", name=f"ps{i}") for i in range(bpc)
            ]

            def emit_compute(it):
                img, j = divmod(it, chunks)
                last = it == nflat - 1
                x, t, o = xb[it % NXB], tb[it % 2], oh[it % 2]
                s, mx, ps = scr[it % 2], m[it % 2], pscr[0]

                # pairwise-max tree; after op 3: s[i] = max over classes
                # {i, i+5, i+10, i+15}; class 20 merges at the end
                nc.vector.tensor_tensor(s[:, 0:5, :], x[:, 0:5, :], x[:, 5:10, :], tt)
                nc.vector.tensor_tensor(s[:, 5:10, :], x[:, 10:15, :], x[:, 15:20, :], tt)
                nc.vector.tensor_tensor(s[:, 0:5, :], s[:, 0:5, :], s[:, 5:10, :], tt)
                nc.vector.tensor_tensor(s[:, 0:2, :], s[:, 0:2, :], s[:, 2:4, :], tt)
                nc.vector.tensor_tensor(s[:, 0, :], s[:, 0, :], s[:, 1, :], tt)
                nc.vector.tensor_tensor(s[:, 0, :], s[:, 0, :], s[:, 4, :], tt)
                nc.vector.tensor_tensor(mx[:], s[:, 0, :], x[:, 20, :], tt)

                if last:
                    # keep the tail on DVE and interleave PE by w-halves
                    for (w0, w1) in ((0, W // 2), (W // 2, W)):
                        mbv = mx[:, w0:w1].unsqueeze(1)
                        nc.vector.tensor_tensor(
                            o[:, 0:C, w0:w1], x[:, 0:C, w0:w1],
                            mbv.broadcast_to((P, C, w1 - w0)), eq,
                        )
                        for g in range(w0, w1):
                            nc.tensor.matmul(
                                psums[img][:, :], t[:, :, g], o[:, :, g],
                                start=(j == 0 and g == 0),
                                stop=(j == chunks - 1 and g == W - 1),
                            )
                else:
                    ndve = C - POOL_EQ
                    mb = mx[:].unsqueeze(1)
                    nc.vector.tensor_tensor(
                        o[:, 0:ndve, :], x[:, 0:ndve, :],
                        mb.broadcast_to((P, ndve, W)), eq,
                    )
                    nc.gpsimd.tensor_tensor(
                        ps[:], x[:, ndve:C, :],
                        mb.broadcast_to((P, POOL_EQ, W)), sub,
                    )
                    nc.gpsimd.tensor_scalar(
                        out=o[:, ndve:C, :], in0=ps[:],
                        scalar1=0.0, scalar2=None, op0=eq,
                    )
                    for g in range(W):
                        nc.tensor.matmul(
                            psums[img][:, :], t[:, :, g], o[:, :, g],
                            start=(j == 0 and g == 0),
                            stop=(j == chunks - 1 and g == W - 1),
                        )
                if j == chunks - 1:
                    nc.vector.tensor_copy(res[img][:], psums[img][:])
                    nc.sync.dma_start(out=stats_ap[img], in_=res[img][:])

            # target DMAs deferred one chunk: the DMA queue runs
            # xb0,xb1,tb0,xb2,tb1,... so the last input lands earlier
            for it in range(nflat):
                img, j = divmod(it, chunks)
                nc.sync.dma_start(out=xb[it % NXB][:], in_=inp_r[img, j])
                if it > 0:
                    im1, jm1 = divmod(it - 1, chunks)
                    nc.sync.dma_start(out=tb[(it - 1) % 2][:, 0:C, :], in_=tgt_r[im1, jm1])
                    emit_compute(it - 1)
            im1, jm1 = divmod(nflat - 1, chunks)
            nc.sync.dma_start(out=tb[(nflat - 1) % 2][:, 0:C, :], in_=tgt_r[im1, jm1])
            emit_compute(nflat - 1)

    return nc


_BUILD_LOCK = threading.Lock()
_NC_CACHE = {}


def get_compiled_nc(key="full"):
    with _BUILD_LOCK:
        if key not in _NC_CACHE:
            nc = bacc.Bacc("TRN2", target_bir_lowering=False, debug=False)
            build_kernel_ir(nc)
            nc.compile()
            _NC_CACHE[key] = nc
        return _NC_CACHE[key]


def combine_stats(stats_all):
    """stats_all: [B, 22, 22] fused psum dumps -> (scores[1:], counts[1:])."""
    M = np.asarray(stats_all, dtype=np.float64)  # [B, 22, 22]
    ci = np.arange(C)
    inter = M[:, ci, ci]  # [B, C] diagonal
    pr = M[:, C, :C]      # [B, C] ones row
    gt = M[:, :C, C]      # [B, C] ones col

    present = (gt > 0).astype(np.float64)
    inter_s = (present * inter).sum(0)
    union_s = (present * (gt + pr)).sum(0) - inter_s + 1e-7
    counts = present.sum(0)
    scores = (inter_s / union_s) * counts
    return scores[1:].astype(np.float32), counts[1:].astype(np.float32)


def prep_inputs(input, target):
    import ml_dtypes

    inp = np.asarray(input, dtype=np.float32)
    tgt = np.asarray(target, dtype=np.float32)
    assert inp.shape == (B, C, H, W), inp.shape
    inp16 = np.ascontiguousarray(inp.astype(np.float16))
    tgt8 = np.ascontiguousarray(tgt.astype(ml_dtypes.float8_e4m3))
    return [
        {
            "input": inp16[i * BPC:(i + 1) * BPC],
            "target": tgt8[i * BPC:(i + 1) * BPC],
        }
        for i in range(NCORES)
    ]


def kernel(input, target):
    in_maps = prep_inputs(input, target)
    nc = get_compiled_nc()
    res = run_bass_kernel_spmd(nc, in_maps, core_ids=list(range(NCORES)))
    stats_all = np.concatenate([r["stats"] for r in res.results], axis=0)  # [B,22,22]
    return combine_stats(stats_all)


if __name__ == "__main__":
    rng = np.random.default_rng(0)
    x = rng.standard_normal((B, C, H, W), dtype=np.float32)
    t = (rng.random((B, C, H, W)) < 0.05).astype(np.float32)
    s, c = kernel(input=x, target=t)
    print("scores:", s)
    print("counts:", c)

